# revision 16
# baseline (speedup 1.0000x reference)
"""Trainium2 Bass kernel for nn_DomainAdapter (moe_routing).

Reference computation (per sample b with expert e = domain_id[b]):
    h  = gelu(x @ down_W.T + down_b)                  # [S, A]
    h  = where(valid, h @ W_e.T + b_e, h) + emb[e]    # per-sample expert
    y  = LayerNorm(h @ up_W.T + up_b + x) * gamma + beta

Strategy (8 NeuronCores, data-parallel over batch; 4 samples/core):
  - host: FUSE the per-sample expert matmul into the up projection:
        M_s = up_W @ W_s            (so expert+up = one [A,D] GEMM)
        c_s = up_W @ (b_s + emb_s) + up_b
    fold c_s into the residual stream (x'' = x + c_s) with a per-sample
    gelu-bias correction db'_s = down_b - down_W @ c_s, so the device
    does exactly two GEMMs per token (down [D->A], fused-up [A->D]).
    Transpose x'' to xT per core, cast the matmul path to bf16.
  - device per 512-token group:
      down:   phT[a,t] = dwT.T @ xT   (ka-outer so gelu overlaps ka=1)
      gelu:   ACT Gelu LUT (erf-exact) + db'_s per-partition bias
      up:     z[t,d] accumulated in PSUM from gt (stationary) x MsT
              (moving), residual added via identity matmuls from xT
      LN:     bn_stats/bn_aggr on DVE; rsqrt via FISR + 1 Newton step,
              batched over ts-pairs on [128,2] tiles; normalize split
              between ACT (scale/bias activation) and DVE (tensor_scalar)
              to balance the two engines; bf16 output
  - ln_gamma/ln_beta applied on host only if non-trivial (they are 1/0)
"""
import numpy as np

from concourse import bacc, tile, mybir
from concourse.bass2jax import (
    _bass_exec_p,
    install_neuronx_cc_hook,
    partition_id_tensor,
)

f32 = mybir.dt.float32
bf16 = mybir.dt.bfloat16
i32 = mybir.dt.int32
AF = mybir.ActivationFunctionType
ALU = mybir.AluOpType

B, S, D, A, E = 32, 1024, 1024, 256, 16
N_CORES = 8
NS = B // N_CORES          # samples per core = 4
T = NS * S                 # tokens per core = 4096
GT = 512                   # tokens per group
NG = T // GT               # 8 groups
TS = 4                     # 128-token sub-tiles per group
KD = D // 128              # 8
KA = A // 128              # 2
DH = D // 512              # 2
LN_EPS = 1e-5
FISR_C = 0x5F3759DF
DVE_NORM_HALVES = 0        # of the 8 (ts,dh) normalize halves per group,
                           # how many run on DVE instead of ACT


def _build_nc(repeat=1):
    nc = bacc.Bacc("TRN2", target_bir_lowering=False, debug=False)

    XT = nc.dram_tensor("xt", [NG, 128, KD, GT], bf16, kind="ExternalInput").ap()
    DWT = nc.dram_tensor("dwt", [D, A], bf16, kind="ExternalInput").ap()
    DB = nc.dram_tensor("db", [128, NS * KA], f32, kind="ExternalInput").ap()
    MST = nc.dram_tensor("mst", [NS, A, D], bf16, kind="ExternalInput").ap()
    IDENT = nc.dram_tensor("ident", [128, 128], bf16, kind="ExternalInput").ap()
    Y = nc.dram_tensor("y", [T, D], bf16, kind="ExternalOutput").ap()

    with tile.TileContext(nc) as tc:
        with (
            tc.tile_pool(name="consts", bufs=1) as cpool,
            tc.tile_pool(name="xTp", bufs=4) as xT_pool,
            tc.tile_pool(name="gtp", bufs=2) as gt_pool,
            tc.tile_pool(name="outp", bufs=6) as out_pool,
            # small LN tiles: ~8 allocs per ts-pair; deep pool so pair N's
            # chain never waits on pair N-1's norm readers
            tc.tile_pool(name="stats", bufs=40) as st_pool,
            tc.tile_pool(name="php", bufs=2, space="PSUM") as ph_pool,
            tc.tile_pool(name="pxp", bufs=6, space="PSUM") as px_pool,
        ):
            dwt = cpool.tile([128, KD, A], bf16, tag="dwt")
            dwt_src = DWT.rearrange("(k p) a -> p k a", p=128)
            nc.sync.dma_start(dwt[:, 0:2], dwt_src[:, 0:2])
            nc.sync.dma_start(dwt[:, 2:KD], dwt_src[:, 2:KD])
            db = cpool.tile([128, NS * KA], f32, tag="db")
            nc.scalar.dma_start(db[:], DB)
            ident = cpool.tile([128, 128], bf16, tag="ident")
            nc.scalar.dma_start(ident[:], IDENT)
            mst = cpool.tile([128, NS, KA, D], bf16, tag="mst")
            mst_src = MST.rearrange("s (k p) d -> p s k d", p=128)
            nc.sync.dma_start(mst[:, 0:2], mst_src[:, 0:2])
            nc.sync.dma_start(mst[:, 2:NS], mst_src[:, 2:NS])

            state = {}

            def dma_front(rep, g):
                xTg = xT_pool.tile([128, KD, GT], bf16, tag="xTg",
                                   name=f"xTg_{rep}_{g}")
                if rep == 0 and g == 0:
                    # split the very first load per contraction chunk so the
                    # first down matmuls start ~3.5us earlier
                    for kd in range(KD):
                        nc.gpsimd.dma_start(xTg[:, kd:kd + 1], XT[g, :, kd:kd + 1])
                else:
                    nc.gpsimd.dma_start(xTg[:], XT[g])
                state[("x", rep, g)] = xTg

            def down_mms(rep, g):
                xTg = state[("x", rep, g)]
                phs = []
                for ka in range(KA):
                    ph = ph_pool.tile([128, GT], f32, tag="ph",
                                      name=f"ph_{rep}_{g}_{ka}")
                    for kd in range(KD):
                        nc.tensor.matmul(
                            ph[:],
                            dwt[:, kd, ka * 128:(ka + 1) * 128],
                            xTg[:, kd, :],
                            start=(kd == 0), stop=(kd == KD - 1),
                        )
                    phs.append(ph)
                state[("ph", rep, g)] = phs

            def gelu(rep, g):
                s = g // 2
                phs = state.pop(("ph", rep, g))
                gt_ = gt_pool.tile([128, KA, GT], bf16, tag="gt",
                                   name=f"gt_{rep}_{g}")
                for ka in range(KA):
                    nc.scalar.activation(
                        gt_[:, ka, :], phs[ka][:], AF.Gelu,
                        bias=db[:, s * KA + ka:s * KA + ka + 1],
                    )
                state[("gt", rep, g)] = gt_

            def back_ts(rep, g, ts):
                """One 128-token block: up+resid matmuls, stats, per-ts FISR
                chain, per-half norms, store. Single-bank psum tiles and
                per-half norms keep the free granularity fine so the scalar
                convoy pipelines across ts blocks."""
                s = g // 2
                xTg = state[("x", rep, g)]
                gt_ = state[("gt", rep, g)]
                pxs = [px_pool.tile([128, 512], f32, tag="px",
                                    name=f"px_{rep}_{g}_{ts}_{dh}")
                       for dh in range(DH)]
                # fused expert+up: gt stationary, MsT moving
                for ka in range(KA):
                    for dh in range(DH):
                        nc.tensor.matmul(
                            pxs[dh][:],
                            gt_[:, ka, ts * 128:(ts + 1) * 128],
                            mst[:, s, ka, dh * 512:(dh + 1) * 512],
                            start=(ka == 0), stop=False,
                        )
                # residual: accumulate x'' via identity matmuls; issue each
                # bank's bn_stats as soon as that bank's accumulation stops
                st = st_pool.tile([128, 12], f32, tag="st",
                                  name=f"st_{rep}_{g}_{ts}")
                for dh in range(DH):
                    for j in range(4):
                        kd = dh * 4 + j
                        nc.tensor.matmul(
                            pxs[dh][:, j * 128:(j + 1) * 128],
                            xTg[:, kd, ts * 128:(ts + 1) * 128],
                            ident[:],
                            start=False, stop=(j == 3),
                        )
                    nc.vector.bn_stats(st[:, dh * 6:(dh + 1) * 6], pxs[dh][:])
                mv = st_pool.tile([128, 2], f32, tag="mv",
                                  name=f"mv_{rep}_{g}_{ts}")
                nc.vector.bn_aggr(mv[:], st[:])

                # FISR rsqrt + 1 Newton step
                varv = mv[:, 1:2]
                vhn = st_pool.tile([128, 1], f32, tag="vhn",
                                   name=f"vhn_{rep}_{g}_{ts}")
                nc.vector.tensor_scalar(vhn[:], varv, -0.5, -0.5 * LN_EPS,
                                        ALU.mult, ALU.add)
                yj = st_pool.tile([128, 1], i32, tag="yj",
                                  name=f"yj_{rep}_{g}_{ts}")
                nc.vector.tensor_scalar(yj[:], varv.bitcast(i32), 1, None,
                                        ALU.logical_shift_right)
                rs = st_pool.tile([128, 1], f32, tag="rs",
                                  name=f"rs_{rep}_{g}_{ts}")
                nc.vector.tensor_scalar(rs[:].bitcast(i32), yj[:], -1, FISR_C,
                                        ALU.mult, ALU.add)
                q = st_pool.tile([128, 1], f32, tag="q",
                                 name=f"q_{rep}_{g}_{ts}")
                tt = st_pool.tile([128, 1], f32, tag="tt",
                                  name=f"tt_{rep}_{g}_{ts}")
                nc.vector.tensor_mul(q[:], rs[:], rs[:])
                nc.vector.tensor_mul(tt[:], q[:], vhn[:])
                nc.vector.scalar_tensor_tensor(
                    rs[:], tt[:], 1.5, rs[:], ALU.add, ALU.mult)
                nmr = st_pool.tile([128, 1], f32, tag="nmr",
                                   name=f"nmr_{rep}_{g}_{ts}")
                nc.vector.scalar_tensor_tensor(
                    nmr[:], mv[:, 0:1], -1.0, rs[:],
                    ALU.mult, ALU.mult)

                outt = out_pool.tile([128, D], bf16, tag="outt",
                                     name=f"outt_{rep}_{g}_{ts}")
                for dh in range(DH):
                    if dh < DVE_NORM_HALVES:
                        nc.vector.tensor_scalar(
                            outt[:, dh * 512:(dh + 1) * 512], pxs[dh][:],
                            rs[:], nmr[:], ALU.mult, ALU.add,
                        )
                    else:
                        nc.scalar.activation(
                            outt[:, dh * 512:(dh + 1) * 512], pxs[dh][:],
                            AF.Identity, bias=nmr[:], scale=rs[:],
                        )
                nc.sync.dma_start(
                    Y[g * GT + ts * 128:g * GT + (ts + 1) * 128, :],
                    outt[:],
                )
                if ts == TS - 1:
                    state.pop(("x", rep, g))
                    state.pop(("gt", rep, g))

            total = repeat * NG
            ix = lambda k: (k // NG, k % NG)
            for k in range(total + 1):
                if k < total:
                    dma_front(*ix(k))
                    down_mms(*ix(k))
                    gelu(*ix(k))
                if k >= 1:
                    for ts in range(TS):
                        back_ts(*ix(k - 1), ts)

    nc.compile()
    return nc


class _Runner:
    """jit-once PJRT runner for the SPMD kernel (axon path)."""

    def __init__(self, nc, n_cores):
        import jax
        from jax.sharding import Mesh, PartitionSpec
        from jax.experimental.shard_map import shard_map

        install_neuronx_cc_hook()
        self.nc = nc
        self.n_cores = n_cores
        pname = nc.partition_id_tensor.name if nc.partition_id_tensor else None

        in_names, out_names, out_avals, zero_outs = [], [], [], []
        for alloc in nc.m.functions[0].allocations:
            if not isinstance(alloc, mybir.MemoryLocationSet):
                continue
            name = alloc.memorylocations[0].name
            if alloc.kind == "ExternalInput":
                if name != pname:
                    in_names.append(name)
            elif alloc.kind == "ExternalOutput":
                out_names.append(name)
                shape = tuple(alloc.tensor_shape)
                dtype = mybir.dt.np(alloc.dtype)
                out_avals.append(jax.core.ShapedArray(shape, dtype))
                zero_outs.append(np.zeros(shape, dtype))
        self.in_names = in_names
        self.out_names = out_names
        self.zero_outs = zero_outs
        n_params = len(in_names)
        n_outs = len(out_avals)
        all_in = list(in_names) + list(out_names)
        if pname is not None:
            all_in.append(pname)

        def _body(*args):
            operands = list(args)
            if pname is not None:
                operands.append(partition_id_tensor())
            outs = _bass_exec_p.bind(
                *operands,
                out_avals=tuple(out_avals),
                in_names=tuple(all_in),
                out_names=tuple(out_names),
                lowering_input_output_aliases=(),
                sim_require_finite=True,
                sim_require_nnan=True,
                nc=nc,
            )
            return tuple(outs)

        devices = jax.devices()[:n_cores]
        mesh = Mesh(np.asarray(devices), ("core",))
        in_specs = (PartitionSpec("core"),) * (n_params + n_outs)
        out_specs = (PartitionSpec("core"),) * n_outs
        self._fn = jax.jit(
            shard_map(_body, mesh=mesh, in_specs=in_specs,
                      out_specs=out_specs, check_rep=False),
            keep_unused=True,
        )

    def run_concat(self, concat_map):
        """concat_map: name -> np array with per-core blocks stacked on axis 0."""
        args = [concat_map[k] for k in self.in_names]
        zeros = [np.concatenate([z] * self.n_cores, axis=0) for z in self.zero_outs]
        outs = self._fn(*args, *zeros)
        return {name: np.asarray(o) for name, o in zip(self.out_names, outs)}


_RUNNER_CACHE = {}


def _get_runner(repeat=1):
    key = repeat
    if key not in _RUNNER_CACHE:
        _RUNNER_CACHE[key] = _Runner(_build_nc(repeat=repeat), N_CORES)
    return _RUNNER_CACHE[key]


def _prep_concat(hidden_states, domain_id, down_W, down_b, up_W, up_b,
                 expert_W, expert_b, domain_emb):
    import ml_dtypes
    bf = ml_dtypes.bfloat16

    hs = np.asarray(hidden_states, dtype=np.float32)
    dom = np.asarray(domain_id)
    valid = (dom >= 0) & (dom < E)
    idx = np.clip(dom, 0, E - 1).astype(np.int64)

    down_W = np.asarray(down_W, dtype=np.float32)
    down_b = np.asarray(down_b, dtype=np.float32)
    up_W = np.asarray(up_W, dtype=np.float32)
    up_b = np.asarray(up_b, dtype=np.float32)
    expert_W = np.asarray(expert_W, dtype=np.float32)
    expert_b = np.asarray(expert_b, dtype=np.float32)
    domain_emb = np.asarray(domain_emb, dtype=np.float32)

    # fuse expert+up per expert: MT_e = (up_W @ W_e).T = W_e.T @ up_W.T
    # bias: c_e = up_W @ (b_e + emb_e) + up_b
    mt_valid = np.einsum('eoa,do->ead', expert_W, up_W)      # [E, A, D]
    c_valid = np.einsum('do,eo->ed', up_W, expert_b + domain_emb) + up_b
    mt_inv = np.ascontiguousarray(up_W.T)                    # [A, D]
    c_inv = domain_emb @ up_W.T + up_b                       # [E, D]

    mst = np.empty((B, A, D), bf)
    cb = np.empty((B, D), np.float32)
    for b in range(B):
        if valid[b]:
            mst[b] = mt_valid[idx[b]].astype(bf)
            cb[b] = c_valid[idx[b]]
        else:
            mst[b] = mt_inv.astype(bf)
            cb[b] = c_inv[idx[b]]

    # per-sample gelu bias correction: the down projection consumes
    # x'' = x + c_b, so db'_b = down_b - down_W @ c_b restores h1
    dbp = down_b[None, :] - cb @ down_W.T                    # [B, A]

    # transposed (x + c_b) per core, group-major so each per-group DMA reads
    # 8KB-contiguous partition rows: [NG, 128p, KD, GT], stacked on axis 0
    hs2 = (hs + cb[:, None, :]).reshape(N_CORES, T, D)
    xt = np.empty((N_CORES * NG, 128, KD, GT), bf)
    for c in range(N_CORES):
        xT = hs2[c].T.astype(bf)  # [D, T]
        xt[c * NG:(c + 1) * NG] = (
            xT.reshape(KD, 128, NG, GT).transpose(2, 1, 0, 3))

    dwT = np.ascontiguousarray(down_W.T).astype(bf)
    ident = np.eye(128, dtype=bf)
    # db tile per core: [128, NS*KA] with db[p, s*KA+ka] = dbp[b, ka*128+p]
    db_t = np.stack([
        np.ascontiguousarray(
            dbp[c * NS:(c + 1) * NS].reshape(NS * KA, 128).T)
        for c in range(N_CORES)
    ])  # [8, 128, NS*KA]

    concat = {
        "xt": xt,
        "dwt": np.concatenate([dwT] * N_CORES, axis=0),
        "db": db_t.reshape(N_CORES * 128, NS * KA),
        "mst": mst.reshape(N_CORES * NS, A, D),
        "ident": np.concatenate([ident] * N_CORES, axis=0),
    }
    return concat


def kernel(hidden_states, domain_id, down_W, down_b, up_W, up_b,
           expert_W, expert_b, domain_emb, ln_gamma, ln_beta):
    concat = _prep_concat(hidden_states, domain_id, down_W, down_b,
                          up_W, up_b, expert_W, expert_b, domain_emb)
    runner = _get_runner()
    outs = runner.run_concat(concat)
    y = outs["y"].astype(np.float32).reshape(B, S, D)

    g = np.asarray(ln_gamma, dtype=np.float32)
    bta = np.asarray(ln_beta, dtype=np.float32)
    if not (np.all(g == 1.0) and np.all(bta == 0.0)):
        y = y * g + bta
    return y
